# revision 31
# baseline (speedup 1.0000x reference)
"""Trainium2 Bass kernel for nn_AttentionModel (S=2048, B=32, H=1024).

Math: reference computes
    energy[b,s] = (enc[s,b,:] @ We.T + (h @ Wh.T + bias)) @ v  ; out = softmax_s(energy)
Since softmax is shift-invariant and the (h @ Wh.T + bias) @ v term is constant
over s, the output reduces exactly to
    out[b, 0, s] = softmax_s( enc[s,b,:] . u ),   u = v[0] @ We   (We = attn_W[:, H:])
So the kernel is a memory-bound [S*B, H] x [H] matvec + row softmax.

Sharding: data-parallel over batch B across 8 cores (4 batches/core).
Device layout per core: enc [BL, H, S] (h on SBUF partitions, s on free dim),
PE matmul contracts h in chunks of 128 (lhsT = u chunk [128,1], rhs = enc tile
[128,512], fp32r, PSUM-accumulated), softmax on ScalarE/VectorE.
"""

import numpy as np

import concourse.bass as bass
import concourse.tile as tile
from concourse import bacc, mybir
from concourse.bass_utils import run_bass_kernel_spmd

S, B, H = 2048, 32, 1024
NCORES = 8
BL = B // NCORES  # batches per core
MM_N = 512        # matmul moving free dim (fp32 max, 1 PSUM bank)


def build_nc(bl=BL, h=H, s=S, enc_bufs=3, jpd=4, use_f32r=True, debug=False,
             taper=True):
    """Build the per-core Bass program (SPMD: same program, different data)."""
    nc = bacc.Bacc()
    f32 = mybir.dt.float32
    jc = h // 128      # h chunks (contraction tiles)
    ns = s // MM_N     # matmul slices per output row
    jpd = min(jpd, jc) # h-chunks per DMA
    nd = jc // jpd     # DMAs per batch
    # Per-batch DMA chunking (in h-chunks of 128). Large chunks sustain the
    # best HBM rate; the last batch tapers so the cold-PE tail after the
    # final chunk is only a few matmuls.
    plan = [[jpd] * nd for _ in range(bl)]
    if taper and jc == 8 and jpd == 4:
        plan[bl - 1] = [4, 2, 1, 1]

    mm_dt = mybir.dt.float32r if use_f32r else f32
    enc_d = nc.declare_dram_parameter("enc", [bl, h, s], mm_dt, isOutput=False)
    u_d = nc.declare_dram_parameter("u", [128, jc], mm_dt, isOutput=False)
    # out rows are UNNORMALIZED exp(e - max); per-row sums go to sums_d and
    # the host divides during the gather (the heavy work - energy matvec,
    # max, exp, sum - all happens on device).
    out_d = nc.declare_dram_parameter("out", [bl, s], f32, isOutput=True)
    sums_d = nc.declare_dram_parameter("sums", [bl, 1], f32, isOutput=True)
    if debug:
        dbg_e = nc.declare_dram_parameter("dbg_e", [bl, s], f32, isOutput=True)
        dbg_p = nc.declare_dram_parameter("dbg_p", [bl, s], f32, isOutput=True)
        dbg_m = nc.declare_dram_parameter("dbg_m", [bl, 4], f32, isOutput=True)

    # Bacc's compile() legalizes multi-semaphore waits (splitting them into
    # EventSemaphore chains), so the structure below can stay simple. Big enc
    # loads go via HWDGE (nc.sync); small transfers via SWDGE (nc.gpsimd).
    with tile.TileContext(nc) as tc:
        with (
            tc.tile_pool(name="up", bufs=1) as up,
            tc.tile_pool(name="encp", bufs=enc_bufs) as encp,
            tc.tile_pool(name="smp", bufs=2) as smp,
            tc.tile_pool(name="op", bufs=1) as op,
            tc.tile_pool(name="psp", bufs=2, space="PSUM") as psp,
        ):
            # Issue the first enc load before anything else so the DMA
            # pipeline starts immediately; the tiny u load follows it.
            t0 = encp.tile([128, plan[0][0], s], mm_dt, name="t",
                           padded_shape=[128, jpd, s])
            nc.sync.dma_start(
                t0[:],
                enc_d[0, 0:plan[0][0] * 128, :].rearrange("(j p) s -> p j s", p=128),
            )
            u_sb = up.tile([128, jc], mm_dt)
            nc.sync.dma_start(u_sb[:], u_d[:])

            o_all = op.tile([1, bl, 1], f32)
            for b in range(bl):
                # Accumulate this batch's energy row in PSUM [1, s] (4 banks,
                # partition 0); 8 fp32r matmuls per 512-wide slice.
                e_ps = psp.tile([1, s], f32)
                j = 0
                for d, cw in enumerate(plan[b]):
                    if b == 0 and d == 0:
                        t = t0
                    else:
                        t = encp.tile([128, cw, s], mm_dt, name="t",
                                      padded_shape=[128, jpd, s])
                        src = enc_d[b, j * 128:(j + cw) * 128, :]
                        nc.sync.dma_start(
                            t[:], src.rearrange("(j p) s -> p j s", p=128)
                        )
                    for jl in range(cw):
                        for ss in range(ns):
                            nc.tensor.matmul(
                                e_ps[:, ss * MM_N:(ss + 1) * MM_N],
                                u_sb[:, j:j + 1],
                                t[:, jl, ss * MM_N:(ss + 1) * MM_N],
                                start=(j == 0),
                                stop=(j == jc - 1),
                            )
                        j += 1
                # Row softmax numerator directly from PSUM; division on host.
                neg_m = smp.tile([1, 1], f32)
                nc.vector.reduce_max(
                    neg_m[:], e_ps[:], axis=mybir.AxisListType.X, negate=True
                )
                p_exp = smp.tile([1, s], f32)
                s_sum = smp.tile([1, 1], f32)
                nc.scalar.activation(
                    p_exp[:], e_ps[:], mybir.ActivationFunctionType.Exp,
                    bias=neg_m[:], accum_out=s_sum[:],
                )
                nc.gpsimd.dma_start(out_d[b:b + 1, :], p_exp[:])
                nc.vector.tensor_copy(o_all[:, b, 0:1], s_sum[:])
                if debug:
                    e_dbg = smp.tile([1, s], f32)
                    nc.scalar.copy(e_dbg[:], e_ps[:])
                    nc.gpsimd.dma_start(dbg_e[b:b + 1, :], e_dbg[:])
                    nc.gpsimd.dma_start(dbg_p[b:b + 1, :], p_exp[:])
                    m_dbg = smp.tile([1, 4], f32)
                    nc.vector.tensor_copy(m_dbg[:, 0:1], neg_m[:])
                    nc.vector.tensor_copy(m_dbg[:, 1:2], s_sum[:])
                    nc.gpsimd.dma_start(dbg_m[b:b + 1, 0:2], m_dbg[:, 0:2])
            # Keep the partition dim explicit on the SBUF side: o_all[0]
            # would make the free dim `bl` look like a partition dim.
            nc.gpsimd.dma_start(sums_d[:], o_all[0:1, :, :])
    nc.compile()
    return nc


def _prep_inputs(encoder_outputs, attn_W, v):
    encoder_outputs = np.asarray(encoder_outputs, dtype=np.float32)
    attn_W = np.asarray(attn_W, dtype=np.float32)
    v = np.asarray(v, dtype=np.float32)
    h = attn_W.shape[0]
    # u = v[0] @ We in float64 (host-side, tiny)
    u = (v[0].astype(np.float64) @ attn_W[:, h:].astype(np.float64)).astype(np.float32)
    u128 = np.ascontiguousarray(u.reshape(h // 128, 128).T)  # [128, jc]
    in_maps = []
    for c in range(NCORES):
        sl = encoder_outputs[:, c * BL:(c + 1) * BL, :]
        enc_c = np.ascontiguousarray(sl.transpose(1, 2, 0))  # [BL, H, S]
        in_maps.append({"enc": enc_c, "u": u128})
    return in_maps


def run(encoder_outputs, rnn_hidden, attn_W, attn_b, v, trace=False, **bass_kwargs):
    in_maps = _prep_inputs(encoder_outputs, attn_W, v)
    nc = build_nc()
    res = run_bass_kernel_spmd(
        nc, in_maps, list(range(NCORES)), trace=trace, **bass_kwargs
    )
    num = np.concatenate([r["out"] for r in res.results], axis=0)    # [B, S]
    sums = np.concatenate([r["sums"] for r in res.results], axis=0)  # [B, 1]
    out = num / sums
    return out[:, None, :].astype(np.float32), res


def kernel(encoder_outputs, rnn_hidden, attn_W, attn_b, v):
    out, _ = run(encoder_outputs, rnn_hidden, attn_W, attn_b, v)
    return out


# revision 33
# speedup vs baseline: 1.0434x; 1.0434x over previous
"""Trainium2 Bass kernel for nn_AttentionModel (S=2048, B=32, H=1024).

Math: reference computes
    energy[b,s] = (enc[s,b,:] @ We.T + (h @ Wh.T + bias)) @ v  ; out = softmax_s(energy)
Since softmax is shift-invariant and the (h @ Wh.T + bias) @ v term is constant
over s, the output reduces exactly to
    out[b, 0, s] = softmax_s( enc[s,b,:] . u ),   u = v[0] @ We   (We = attn_W[:, H:])
So the kernel is a memory-bound [S*B, H] x [H] matvec + row softmax.

Sharding: data-parallel over batch B across 8 cores (4 batches/core).
Device layout per core: enc [BL, H, S] (h on SBUF partitions, s on free dim),
PE matmul contracts h in chunks of 128 (lhsT = u chunk [128,1], rhs = enc tile
[128,512], fp32r, PSUM-accumulated), softmax on ScalarE/VectorE.
"""

import numpy as np

import concourse.bass as bass
import concourse.tile as tile
from concourse import bacc, mybir
from concourse.bass_utils import run_bass_kernel_spmd

S, B, H = 2048, 32, 1024
NCORES = 8
BL = B // NCORES  # batches per core
MM_N = 512        # matmul moving free dim (fp32 max, 1 PSUM bank)


def build_nc(bl=BL, h=H, s=S, enc_bufs=4, jpd=4, use_f32r=True, debug=False,
             taper=True):
    """Build the per-core Bass program (SPMD: same program, different data)."""
    nc = bacc.Bacc()
    f32 = mybir.dt.float32
    jc = h // 128      # h chunks (contraction tiles)
    ns = s // MM_N     # matmul slices per output row
    jpd = min(jpd, jc) # h-chunks per DMA
    nd = jc // jpd     # DMAs per batch
    # Per-batch DMA chunking (in h-chunks of 128). Large chunks sustain the
    # best HBM rate; the last batch tapers so the cold-PE tail after the
    # final chunk is only a few matmuls.
    plan = [[jpd] * nd for _ in range(bl)]
    if taper and jc == 8 and jpd == 4:
        plan[bl - 1] = [4, 2, 1, 1]

    mm_dt = mybir.dt.float32r if use_f32r else f32
    enc_d = nc.declare_dram_parameter("enc", [bl, h, s], mm_dt, isOutput=False)
    u_d = nc.declare_dram_parameter("u", [128, jc], mm_dt, isOutput=False)
    # out rows are UNNORMALIZED exp(e - max); per-row sums go to sums_d and
    # the host divides during the gather (the heavy work - energy matvec,
    # max, exp, sum - all happens on device).
    out_d = nc.declare_dram_parameter("out", [bl, s], f32, isOutput=True)
    sums_d = nc.declare_dram_parameter("sums", [bl, 1], f32, isOutput=True)
    if debug:
        dbg_e = nc.declare_dram_parameter("dbg_e", [bl, s], f32, isOutput=True)
        dbg_p = nc.declare_dram_parameter("dbg_p", [bl, s], f32, isOutput=True)
        dbg_m = nc.declare_dram_parameter("dbg_m", [bl, 4], f32, isOutput=True)

    # Bacc's compile() legalizes multi-semaphore waits (splitting them into
    # EventSemaphore chains), so the structure below can stay simple. Big enc
    # loads go via HWDGE (nc.sync); small transfers via SWDGE (nc.gpsimd).
    with tile.TileContext(nc) as tc:
        with (
            tc.tile_pool(name="up", bufs=1) as up,
            tc.tile_pool(name="encp", bufs=enc_bufs) as encp,
            tc.tile_pool(name="smp", bufs=2) as smp,
            tc.tile_pool(name="op", bufs=1) as op,
            tc.tile_pool(name="psp", bufs=2, space="PSUM") as psp,
        ):
            # Issue the first enc load before anything else so the DMA
            # pipeline starts immediately; the tiny u load follows it.
            t0 = encp.tile([128, plan[0][0], s], mm_dt, name="t",
                           padded_shape=[128, jpd, s])
            nc.sync.dma_start(
                t0[:],
                enc_d[0, 0:plan[0][0] * 128, :].rearrange("(j p) s -> p j s", p=128),
            )
            u_sb = up.tile([128, jc], mm_dt)
            nc.sync.dma_start(u_sb[:], u_d[:])

            o_all = op.tile([1, bl, 1], f32)
            for b in range(bl):
                # Accumulate this batch's energy row in PSUM [1, s] (4 banks,
                # partition 0); 8 fp32r matmuls per 512-wide slice.
                e_ps = psp.tile([1, s], f32)
                m4 = smp.tile([1, ns], f32)
                j = 0
                for d, cw in enumerate(plan[b]):
                    if b == 0 and d == 0:
                        t = t0
                    else:
                        t = encp.tile([128, cw, s], mm_dt, name="t",
                                      padded_shape=[128, jpd, s])
                        src = enc_d[b, j * 128:(j + cw) * 128, :]
                        nc.sync.dma_start(
                            t[:], src.rearrange("(j p) s -> p j s", p=128)
                        )
                    for jl in range(cw):
                        for ss in range(ns):
                            nc.tensor.matmul(
                                e_ps[:, ss * MM_N:(ss + 1) * MM_N],
                                u_sb[:, j:j + 1],
                                t[:, jl, ss * MM_N:(ss + 1) * MM_N],
                                start=(j == 0),
                                stop=(j == jc - 1),
                            )
                            if j == jc - 1:
                                # Per-slice max as soon as this 512-slice's
                                # accumulation group stops - overlaps the
                                # remaining matmuls (disjoint PSUM banks).
                                nc.vector.reduce_max(
                                    m4[:, ss:ss + 1],
                                    e_ps[:, ss * MM_N:(ss + 1) * MM_N],
                                    axis=mybir.AxisListType.X,
                                )
                        j += 1
                # Row softmax numerator directly from PSUM; division on host.
                neg_m = smp.tile([1, 1], f32)
                nc.vector.reduce_max(
                    neg_m[:], m4[:], axis=mybir.AxisListType.X, negate=True
                )
                p_exp = smp.tile([1, s], f32)
                s_sum = smp.tile([1, 1], f32)
                nc.scalar.activation(
                    p_exp[:], e_ps[:], mybir.ActivationFunctionType.Exp,
                    bias=neg_m[:], accum_out=s_sum[:],
                )
                nc.gpsimd.dma_start(out_d[b:b + 1, :], p_exp[:])
                nc.vector.tensor_copy(o_all[:, b, 0:1], s_sum[:])
                if debug:
                    e_dbg = smp.tile([1, s], f32)
                    nc.scalar.copy(e_dbg[:], e_ps[:])
                    nc.gpsimd.dma_start(dbg_e[b:b + 1, :], e_dbg[:])
                    nc.gpsimd.dma_start(dbg_p[b:b + 1, :], p_exp[:])
                    m_dbg = smp.tile([1, 4], f32)
                    nc.vector.tensor_copy(m_dbg[:, 0:1], neg_m[:])
                    nc.vector.tensor_copy(m_dbg[:, 1:2], s_sum[:])
                    nc.gpsimd.dma_start(dbg_m[b:b + 1, 0:2], m_dbg[:, 0:2])
            # Keep the partition dim explicit on the SBUF side: o_all[0]
            # would make the free dim `bl` look like a partition dim.
            nc.gpsimd.dma_start(sums_d[:], o_all[0:1, :, :])
    nc.compile()
    return nc


def _prep_inputs(encoder_outputs, attn_W, v):
    encoder_outputs = np.asarray(encoder_outputs, dtype=np.float32)
    attn_W = np.asarray(attn_W, dtype=np.float32)
    v = np.asarray(v, dtype=np.float32)
    h = attn_W.shape[0]
    # u = v[0] @ We in float64 (host-side, tiny)
    u = (v[0].astype(np.float64) @ attn_W[:, h:].astype(np.float64)).astype(np.float32)
    u128 = np.ascontiguousarray(u.reshape(h // 128, 128).T)  # [128, jc]
    in_maps = []
    for c in range(NCORES):
        sl = encoder_outputs[:, c * BL:(c + 1) * BL, :]
        enc_c = np.ascontiguousarray(sl.transpose(1, 2, 0))  # [BL, H, S]
        in_maps.append({"enc": enc_c, "u": u128})
    return in_maps


def run(encoder_outputs, rnn_hidden, attn_W, attn_b, v, trace=False, **bass_kwargs):
    in_maps = _prep_inputs(encoder_outputs, attn_W, v)
    nc = build_nc()
    res = run_bass_kernel_spmd(
        nc, in_maps, list(range(NCORES)), trace=trace, **bass_kwargs
    )
    num = np.concatenate([r["out"] for r in res.results], axis=0)    # [B, S]
    sums = np.concatenate([r["sums"] for r in res.results], axis=0)  # [B, 1]
    out = num / sums
    return out[:, None, :].astype(np.float32), res


def kernel(encoder_outputs, rnn_hidden, attn_W, attn_b, v):
    out, _ = run(encoder_outputs, rnn_hidden, attn_W, attn_b, v)
    return out


# revision 34
# speedup vs baseline: 1.2016x; 1.1517x over previous
"""Trainium2 Bass kernel for nn_AttentionModel (S=2048, B=32, H=1024).

Math: reference computes
    energy[b,s] = (enc[s,b,:] @ We.T + (h @ Wh.T + bias)) @ v  ; out = softmax_s(energy)
Since softmax is shift-invariant and the (h @ Wh.T + bias) @ v term is constant
over s, the output reduces exactly to
    out[b, 0, s] = softmax_s( enc[s,b,:] . u ),   u = v[0] @ We   (We = attn_W[:, H:])
So the kernel is a memory-bound [S*B, H] x [H] matvec + row softmax.

Sharding: data-parallel over batch B across 8 cores (4 batches/core).
Device layout per core: enc [BL, H, S] (h on SBUF partitions, s on free dim),
PE matmul contracts h in chunks of 128 (lhsT = u chunk [128,1], rhs = enc tile
[128,512], fp32r, PSUM-accumulated), softmax on ScalarE/VectorE.
"""

import numpy as np

import concourse.bass as bass
import concourse.tile as tile
from concourse import bacc, mybir
from concourse.bass_utils import run_bass_kernel_spmd

S, B, H = 2048, 32, 1024
NCORES = 8
BL = B // NCORES  # batches per core
MM_N = 512        # matmul moving free dim (fp32 max, 1 PSUM bank)


def build_nc(bl=BL, h=H, s=S, enc_bufs=4, jpd=4, use_f32r=True, debug=False,
             taper=True):
    """Build the per-core Bass program (SPMD: same program, different data)."""
    nc = bacc.Bacc()
    f32 = mybir.dt.float32
    jc = h // 128      # h chunks (contraction tiles)
    ns = s // MM_N     # matmul slices per output row
    jpd = min(jpd, jc) # h-chunks per DMA
    nd = jc // jpd     # DMAs per batch
    # Per-batch DMA chunking (in h-chunks of 128). Large chunks sustain the
    # best HBM rate; the last batch tapers so the cold-PE tail after the
    # final chunk is only a few matmuls.
    plan = [[jpd] * nd for _ in range(bl)]
    if taper and jc == 8 and jpd in (4, 8):
        plan[bl - 1] = [4, 2, 1, 1]

    mm_dt = mybir.dt.float32r if use_f32r else f32
    enc_d = nc.declare_dram_parameter("enc", [bl, h, s], mm_dt, isOutput=False)
    u_d = nc.declare_dram_parameter("u", [128, jc], mm_dt, isOutput=False)
    # out rows are UNNORMALIZED exp(e - max); per-row sums go to sums_d and
    # the host divides during the gather (the heavy work - energy matvec,
    # max, exp, sum - all happens on device).
    out_d = nc.declare_dram_parameter("out", [bl, s], f32, isOutput=True)
    sums_d = nc.declare_dram_parameter("sums", [bl, 1], f32, isOutput=True)
    if debug:
        dbg_e = nc.declare_dram_parameter("dbg_e", [bl, s], f32, isOutput=True)
        dbg_p = nc.declare_dram_parameter("dbg_p", [bl, s], f32, isOutput=True)
        dbg_m = nc.declare_dram_parameter("dbg_m", [bl, 4], f32, isOutput=True)

    # Bacc's compile() legalizes multi-semaphore waits (splitting them into
    # EventSemaphore chains), so the structure below can stay simple. Big enc
    # loads go via HWDGE (nc.sync); small transfers via SWDGE (nc.gpsimd).
    with tile.TileContext(nc) as tc:
        with (
            tc.tile_pool(name="up", bufs=1) as up,
            tc.tile_pool(name="encp", bufs=enc_bufs) as encp,
            tc.tile_pool(name="smp", bufs=2) as smp,
            tc.tile_pool(name="op", bufs=1) as op,
            tc.tile_pool(name="psp", bufs=2, space="PSUM") as psp,
        ):
            # Issue the first enc load before anything else so the DMA
            # pipeline starts immediately; the tiny u load follows it.
            t0 = encp.tile([128, plan[0][0], s], mm_dt, name="t",
                           padded_shape=[128, jpd, s])
            nc.sync.dma_start(
                t0[:],
                enc_d[0, 0:plan[0][0] * 128, :].rearrange("(j p) s -> p j s", p=128),
            )
            u_sb = up.tile([128, jc], mm_dt)
            nc.sync.dma_start(u_sb[:], u_d[:])

            o_all = op.tile([1, bl, 1], f32)
            for b in range(bl):
                # Accumulate this batch's energy row in PSUM [1, s] (4 banks,
                # partition 0); 8 fp32r matmuls per 512-wide slice.
                e_ps = psp.tile([1, s], f32)
                m4 = smp.tile([1, ns], f32)
                j = 0
                for d, cw in enumerate(plan[b]):
                    if b == 0 and d == 0:
                        t = t0
                    else:
                        t = encp.tile([128, cw, s], mm_dt, name="t",
                                      padded_shape=[128, jpd, s])
                        src = enc_d[b, j * 128:(j + cw) * 128, :]
                        nc.sync.dma_start(
                            t[:], src.rearrange("(j p) s -> p j s", p=128)
                        )
                    for jl in range(cw):
                        for ss in range(ns):
                            nc.tensor.matmul(
                                e_ps[:, ss * MM_N:(ss + 1) * MM_N],
                                u_sb[:, j:j + 1],
                                t[:, jl, ss * MM_N:(ss + 1) * MM_N],
                                start=(j == 0),
                                stop=(j == jc - 1),
                            )
                            if j == jc - 1:
                                # Per-slice max as soon as this 512-slice's
                                # accumulation group stops - overlaps the
                                # remaining matmuls (disjoint PSUM banks).
                                nc.vector.reduce_max(
                                    m4[:, ss:ss + 1],
                                    e_ps[:, ss * MM_N:(ss + 1) * MM_N],
                                    axis=mybir.AxisListType.X,
                                )
                        j += 1
                # Row softmax numerator directly from PSUM; division on host.
                neg_m = smp.tile([1, 1], f32)
                nc.vector.reduce_max(
                    neg_m[:], m4[:], axis=mybir.AxisListType.X, negate=True
                )
                p_exp = smp.tile([1, s], f32)
                s_sum = smp.tile([1, 1], f32)
                nc.scalar.activation(
                    p_exp[:], e_ps[:], mybir.ActivationFunctionType.Exp,
                    bias=neg_m[:], accum_out=s_sum[:],
                )
                nc.gpsimd.dma_start(out_d[b:b + 1, :], p_exp[:])
                nc.vector.tensor_copy(o_all[:, b, 0:1], s_sum[:])
                if debug:
                    e_dbg = smp.tile([1, s], f32)
                    nc.scalar.copy(e_dbg[:], e_ps[:])
                    nc.gpsimd.dma_start(dbg_e[b:b + 1, :], e_dbg[:])
                    nc.gpsimd.dma_start(dbg_p[b:b + 1, :], p_exp[:])
                    m_dbg = smp.tile([1, 4], f32)
                    nc.vector.tensor_copy(m_dbg[:, 0:1], neg_m[:])
                    nc.vector.tensor_copy(m_dbg[:, 1:2], s_sum[:])
                    nc.gpsimd.dma_start(dbg_m[b:b + 1, 0:2], m_dbg[:, 0:2])
            # Keep the partition dim explicit on the SBUF side: o_all[0]
            # would make the free dim `bl` look like a partition dim.
            nc.gpsimd.dma_start(sums_d[:], o_all[0:1, :, :])
    nc.compile()
    return nc


def _prep_inputs(encoder_outputs, attn_W, v):
    encoder_outputs = np.asarray(encoder_outputs, dtype=np.float32)
    attn_W = np.asarray(attn_W, dtype=np.float32)
    v = np.asarray(v, dtype=np.float32)
    h = attn_W.shape[0]
    # u = v[0] @ We in float64 (host-side, tiny)
    u = (v[0].astype(np.float64) @ attn_W[:, h:].astype(np.float64)).astype(np.float32)
    u128 = np.ascontiguousarray(u.reshape(h // 128, 128).T)  # [128, jc]
    in_maps = []
    for c in range(NCORES):
        sl = encoder_outputs[:, c * BL:(c + 1) * BL, :]
        enc_c = np.ascontiguousarray(sl.transpose(1, 2, 0))  # [BL, H, S]
        in_maps.append({"enc": enc_c, "u": u128})
    return in_maps


def run(encoder_outputs, rnn_hidden, attn_W, attn_b, v, trace=False, **bass_kwargs):
    in_maps = _prep_inputs(encoder_outputs, attn_W, v)
    nc = build_nc()
    res = run_bass_kernel_spmd(
        nc, in_maps, list(range(NCORES)), trace=trace, **bass_kwargs
    )
    num = np.concatenate([r["out"] for r in res.results], axis=0)    # [B, S]
    sums = np.concatenate([r["sums"] for r in res.results], axis=0)  # [B, 1]
    out = num / sums
    return out[:, None, :].astype(np.float32), res


def kernel(encoder_outputs, rnn_hidden, attn_W, attn_b, v):
    out, _ = run(encoder_outputs, rnn_hidden, attn_W, attn_b, v)
    return out
